# revision 19
# baseline (speedup 1.0000x reference)
"""GQA attention layer for Trainium2, tensor-parallel over kv-heads on 8 NeuronCores.

Problem: x:(1,2048,2048) f32, causal mask; q/k/v/o projections with
NUM_HEADS=32, NUM_KV_HEADS=8, HEAD_DIM=128, GROUP=4.

Sharding: core c owns kv-head c and its 4 query heads (columns 4c*128..(4c+4)*128
of wq, rows of wo). Each core computes a partial y_c = attnout_c @ wo_c; the host
sums the 8 partials and adds bo.

Dataflow on each core (all "transposed" so no on-chip transposes of the big
probability matrix are ever needed):
  qT[d,i] = wq_c.T(h-major) accumulation:  matmul(lhsT=wq_kt, rhs=xT_kt)
  kT[d,j], vT[d,j] likewise;  v[j,d] via 16 PE transposes of vT.
  sT[j,i] = matmul(lhsT=kT_jtile, rhs=qT_chunk)   (contraction = head_dim, 1 mm)
  e = exp(sT) on ACT (1/sqrt(d) folded into the qT drain), causal-masked by a
      [128,128] triu multiply on the triangular quarter of diagonal j-tiles.
  colsum: e-tiles are pre-summed on DVE (fp16, 2x rate) into acc; ONE
      PE matmul ones.T @ acc per (h,chunk) yields the softmax denominator
      (the baseline burned one PE matmul per block on this).
  recip = 1/colsum (DVE); broadcast to 128 partitions with a k=1 PE matmul;
  aoT = avpsum * recip (DVE), fp16
  y[i,hid] += aoT_head_tile.T @ wo_head  (4 head k-tiles), fp16 out, DMA to HBM;
  the host sums the 8 fp16 partials in f64 and adds bo.

Causality: for i-chunk c (512 wide) only j-tiles 0..4c+3 are computed, and the
above-diagonal quarters of diagonal j-tiles are skipped (ragged score/av
matmuls over qT[:, i0:]).

Scheduling: the PE issues in order, and every stall resets its p-state ramp
(full 2.4 GHz only after ~3us of continuous execution), so stall-free emission
order matters more than instruction count.  The emitter weaves independent
work into the attention stream via a FIFO fill queue: y-projection tiles of
chunk c-1 and the cs/recip-broadcast chain of the previous head are emitted
between score/av matmuls, and next-chunk projections run right after each
attention chunk.  x is DMA'd in per-chunk pieces so chunk-0 projections start
~2us in instead of waiting for the full x load.
"""

import math
from collections import deque

import numpy as np

HIDDEN = 2048
HEAD_DIM = 128
NUM_HEADS = 32
NUM_KV = 8
GROUP = NUM_HEADS // NUM_KV
S = 2048
NCORES = 8
CH = 512                      # i-chunk width
NCH = S // CH                 # 4 i-chunks
KT = HIDDEN // 128            # 16 contraction tiles over hidden
NJT = S // 128                # 16 j-tiles
INV_SQRT_D = 1.0 / math.sqrt(HEAD_DIM)
LAG = 4                       # score->av lag in j-blocks

# Module-level knobs for test.py (the grading harness uses the defaults).
TRACE = False
LAST_EXEC_NS = None
LAST_RESULTS = None

_PROG_CACHE = {}


def _build(mode):
    """mode: 'causal' (skip upper blocks + ragged diagonals), 'full' (all-ones
    mask), 'generic' (multiplicative fp16 mask tiles from HBM)."""
    import concourse.bacc as bacc
    import concourse.tile as tile
    import concourse.mybir as mybir
    from concourse.masks import make_identity

    f32 = mybir.dt.float32
    f16 = mybir.dt.float16
    Ident = mybir.ActivationFunctionType.Identity
    Exp = mybir.ActivationFunctionType.Exp

    nc = bacc.Bacc(None, target_bir_lowering=False)

    # PK packs, per hidden k-tile, [wq(512) | wk(128) | wv(128) | xA(512)] so
    # chunk-0 projections are fed by ONE dma per k-tile (issue-rate bound
    # otherwise: ~617ns per descriptor on the Sync queue).
    PKW = GROUP * HEAD_DIM + 2 * HEAD_DIM + CH
    pk_d = nc.dram_tensor("pk", [HIDDEN, PKW], f16, kind="ExternalInput")
    xB_d = nc.dram_tensor("xB", [HIDDEN, S - CH], f16, kind="ExternalInput")
    wo_d = nc.dram_tensor("wo", [GROUP * HEAD_DIM, HIDDEN], f16, kind="ExternalInput")
    bias_d = nc.dram_tensor("biasp", [128, 6], f32, kind="ExternalInput")
    if mode == "causal":
        ms_d = nc.dram_tensor("mstrip", [128, 128], f16, kind="ExternalInput")
    if mode == "generic":
        mk_d = nc.dram_tensor("maskT", [S, S], f16, kind="ExternalInput")
    y_d = nc.dram_tensor("y", [S, HIDDEN], f16, kind="ExternalOutput")

    def nblocks(c):
        return 4 * c + 4 if mode == "causal" else NJT

    with tile.TileContext(nc) as tc:
        with (
            tc.tile_pool(name="consts", bufs=1) as consts,
            tc.tile_pool(name="xw", bufs=1) as xw,
            tc.tile_pool(name="proj", bufs=1) as proj,
            tc.tile_pool(name="epool", bufs=6) as epool,
            tc.tile_pool(name="accp", bufs=1) as accp,
            tc.tile_pool(name="rpool", bufs=1) as rpool,
            tc.tile_pool(name="ypool", bufs=6) as ypool,
            tc.tile_pool(name="pp", bufs=4, space="PSUM") as pp,
            tc.tile_pool(name="spp", bufs=2, space="PSUM") as spp,
            tc.tile_pool(name="avp", bufs=1, space="PSUM") as avp,
            tc.tile_pool(name="nrm", bufs=1, space="PSUM") as nrm,
        ):
            # ---- constants ----
            ident = consts.tile([128, 128], f16, tag="ident", name="ident")
            make_identity(nc, ident)
            ones_col = consts.tile([128, 1], f16, tag="ones_col", name="ones_col")
            nc.vector.memset(ones_col, 1.0)
            # ---- input loads (all on Sync, priority order; y writebacks go
            # to the otherwise-idle GpSimd queue).  bias/mstrip are loaded
            # after the PK tiles: they are first needed ~8us later than pk0,
            # which gates the very first matmul. ----
            pk_sb, xB, wo_sb = [], [], []
            bias_sb = mstrip = None
            for kt in range(KT):
                pkt = xw.tile([128, PKW], f16, tag=f"pk{kt}", name=f"pk{kt}")
                nc.sync.dma_start(out=pkt, in_=pk_d[kt * 128:(kt + 1) * 128, :])
                pk_sb.append(pkt)
                if kt == 3:
                    bias_sb = consts.tile([128, 6], f32, tag="bias",
                                          name="bias_sb")
                    nc.sync.dma_start(out=bias_sb, in_=bias_d[:, :])
                    if mode == "causal":
                        mstrip = consts.tile([128, 128], f16, tag="mstrip",
                                             name="mstrip")
                        nc.sync.dma_start(out=mstrip, in_=ms_d[:, :])
            for kt in range(KT):
                xb = xw.tile([128, S - CH], f16, tag=f"xB{kt}", name=f"xB{kt}")
                nc.sync.dma_start(out=xb, in_=xB_d[kt * 128:(kt + 1) * 128, :])
                xB.append(xb)
            for h in range(GROUP):
                wot = xw.tile([128, HIDDEN], f16, tag=f"wo{h}", name=f"wo{h}")
                nc.sync.dma_start(out=wot, in_=wo_d[h * 128:(h + 1) * 128, :])
                wo_sb.append(wot)

            def xr(c, kt):
                if c == 0:
                    return pk_sb[kt][:, 768:768 + CH]
                return xB[kt][:, (c - 1) * CH:c * CH]

            # ---- fill queue: independent PE work woven into stall-prone
            # stretches of the stream.  `reserve` holds back items so the
            # last chunk's softmax-chain latency can be covered at the very
            # end (no next chunk to weave into). ----
            fill = deque()
            reserve = [0]

            def pump(n=1):
                for _ in range(min(n, len(fill) - reserve[0])):
                    fill.popleft()()

            def drain_fill():
                while fill:
                    fill.popleft()()

            kT_c = [None] * NCH
            qT = {}
            v_sb = [None] * NJT
            vT_cur = [None]
            aoT = {}
            mask_sb = {}

            # ---- projections for chunk c (two passes, <=4 PSUM banks) ----
            def proj_pass(c, spec, pumps, fill_rate=0):
                """spec: list of ('k'|'v'|0..3).  pumps: dict kt -> closure."""
                pss = [pp.tile([128, CH], f32, tag="pp", name=f"P{c}{s}")
                       for s in spec]
                for kt in range(KT):
                    for ps, s in zip(pss, spec):
                        if s == "k":
                            lhsT = pk_sb[kt][:, 512:640]
                        elif s == "v":
                            lhsT = pk_sb[kt][:, 640:768]
                        else:
                            lhsT = pk_sb[kt][:, s * 128:(s + 1) * 128]
                        nc.tensor.matmul(ps, lhsT=lhsT, rhs=xr(c, kt),
                                         start=(kt == 0), stop=(kt == KT - 1))
                    if kt in pumps:
                        pumps[kt]()
                    elif fill_rate:
                        pump(fill_rate)
                for ps, s in zip(pss, spec):
                    if s == "k":
                        kt_t = proj.tile([128, CH], f16, tag=f"kT{c}",
                                         name=f"kT{c}")
                        nc.scalar.activation(kt_t, ps, Ident, bias=bias_sb[:, 4:5])
                        kT_c[c] = kt_t
                    elif s == "v":
                        vt_t = proj.tile([128, CH], f16, tag=f"vT{c % 2}",
                                         name=f"vT{c}")
                        nc.scalar.activation(vt_t, ps, Ident, bias=bias_sb[:, 5:6])
                        vT_cur[0] = vt_t
                    else:
                        qt_t = proj.tile([128, CH], f16, tag=f"q{s}_{c % 2}",
                                         name=f"q{s}_{c}")
                        nc.scalar.activation(qt_t, ps, Ident,
                                             bias=bias_sb[:, s:s + 1],
                                             scale=INV_SQRT_D)
                        qT[(s, c)] = qt_t

            def vtrans(c):
                for d in range(4):
                    b = 4 * c + d
                    tp = spp.tile([128, 128], f16, tag="s", name=f"tp{b}")
                    nc.tensor.transpose(tp, vT_cur[0][:, d * 128:(d + 1) * 128],
                                        ident)
                    vt = proj.tile([128, 128], f16, tag=f"v{b}", name=f"v{b}")
                    nc.vector.tensor_copy(vt, tp)
                    v_sb[b] = vt

            # ---- attention for one (chunk, head) ----
            def attn_head(c, h, chain):
                nb = nblocks(c)
                qt = qT[(h, c)]
                par = (h + c) % 2
                acc = accp.tile([128, CH], f16, tag=f"acc{par}",
                                name=f"acc{h}_{c}")
                av = avp.tile([128, CH], f32, tag="av", name=f"av{h}_{c}")
                pend = deque()

                def emit_av():
                    b, i0, e = pend.popleft()
                    nc.tensor.matmul(av[:, i0:], lhsT=v_sb[b], rhs=e[:, i0:],
                                     start=(b == 0), stop=(b == nb - 1),
                                     skip_group_check=True)

                for b in range(nb):
                    diag = mode == "causal" and b >= 4 * c
                    i0 = 128 * (b - 4 * c) if diag else 0
                    sp = spp.tile([128, CH], f32, tag="s", name=f"s{h}_{c}_{b}")
                    nc.tensor.matmul(sp[:, i0:],
                                     lhsT=kT_c[b // 4][:, (b % 4) * 128:
                                                       (b % 4 + 1) * 128],
                                     rhs=qt[:, i0:], start=True, stop=True)
                    e = epool.tile([128, CH], f16, tag="e", name=f"e{h}_{c}_{b}")
                    nc.scalar.activation(e[:, i0:], sp[:, i0:], Exp)
                    if diag:
                        nc.vector.tensor_mul(e[:, i0:i0 + 128],
                                             e[:, i0:i0 + 128], mstrip)
                    elif mode == "generic":
                        nc.vector.tensor_mul(e, e, mask_sb[b])
                    if b == 0:
                        nc.vector.tensor_copy(acc, e)
                    else:
                        nc.vector.tensor_add(acc[:, i0:], acc[:, i0:],
                                             e[:, i0:])
                    pend.append((b, i0, e))
                    if chain is not None and b == 0:
                        chain[0]()
                    elif chain is not None and b == 2:
                        chain[1]()
                        pump(1)
                    else:
                        pump(1)
                    if len(pend) > LAG:
                        emit_av()
                while pend:
                    emit_av()

                cell = {}

                def cs_fn():
                    cs = nrm.tile([1, CH], f32, tag="n", name=f"cs{h}_{c}")
                    nc.tensor.matmul(cs, lhsT=ones_col, rhs=acc,
                                     start=True, stop=True)
                    rc = rpool.tile([1, CH], f32, tag=f"rc{par}",
                                    name=f"rc{h}_{c}")
                    nc.vector.reciprocal_approx_fast(rc, cs)
                    cell["rc"] = rc

                def rb_fn():
                    # partition-broadcast the reciprocal off the PE (the
                    # baseline burned a k=1 matmul + drain copy on this)
                    rb = rpool.tile([128, CH], f32, tag=f"rb{par}",
                                    name=f"rb{h}_{c}")
                    nc.gpsimd.partition_broadcast(rb, cell["rc"], 128)
                    ao = proj.tile([128, CH], f16, tag=f"ao{h}_{c % 2}",
                                   name=f"ao{h}_{c}")
                    nc.vector.tensor_mul(ao, av, rb)
                    aoT[(h, c)] = ao

                return (cs_fn, rb_fn)

            # ---- y projection fill items (one per (it, nh, h) matmul so the
            # A-phase can pump exactly one PE op per attention block).
            # Writebacks: chunks 0-2 batch 4 column tiles into one [128,2048]
            # DMA per row-tile; the last chunk uses per-column DMAs to keep
            # the tail short.  All y DMAs ride the GpSimd queue. ----
            def make_yh(c, it, nh, h, cell):
                def f():
                    if h == 0:
                        cell["yp"] = pp.tile([128, CH], f32, tag="pp",
                                             name=f"yp{c}_{it}_{nh}")
                    yp = cell["yp"]
                    nc.tensor.matmul(
                        yp, lhsT=aoT[(h, c)][:, it * 128:(it + 1) * 128],
                        rhs=wo_sb[h][:, nh * CH:(nh + 1) * CH],
                        start=(h == 0), stop=(h == GROUP - 1))
                    if h != GROUP - 1:
                        return
                    r0 = c * CH + it * 128
                    if c == NCH - 1:
                        ysb = ypool.tile([128, CH], f16, tag="y",
                                         name=f"y{c}_{it}_{nh}")
                        nc.scalar.copy(ysb, yp)
                        nc.gpsimd.dma_start(
                            out=y_d[r0:r0 + 128, nh * CH:(nh + 1) * CH],
                            in_=ysb)
                    else:
                        if nh == 0:
                            cell["ysb"] = ypool.tile([128, HIDDEN], f16,
                                                     tag="yb",
                                                     name=f"yb{c}_{it}")
                        ysb = cell["ysb"]
                        nc.vector.tensor_copy(
                            ysb[:, nh * CH:(nh + 1) * CH], yp)
                        if nh == NCH - 1:
                            nc.gpsimd.dma_start(out=y_d[r0:r0 + 128, :],
                                                in_=ysb)
                return f

            def enqueue_y(c):
                for it in range(CH // 128):
                    cell = {}
                    for nh in range(NCH):
                        for h in range(GROUP):
                            fill.append(make_yh(c, it, nh, h, cell))

            def load_masks(c):
                for b in range(nblocks(c)):
                    mt = proj.tile([128, CH], f16, tag=f"m{b}", name=f"m{b}_{c}")
                    nc.sync.dma_start(
                        out=mt,
                        in_=mk_d[b * 128:(b + 1) * 128, c * CH:(c + 1) * CH])
                    mask_sb[b] = mt

            # ---- top-level emission ----
            chain = None
            if mode == "causal":
                for c in range(NCH):
                    p1 = {2: chain[0]} if chain is not None else {}
                    p2 = {0: chain[1]} if chain is not None else {}
                    proj_pass(c, ["k", "v", 0, 1], p1)
                    vtrans(c)
                    proj_pass(c, [2, 3], p2, fill_rate=1)
                    chain = None
                    if c < NCH - 1:
                        drain_fill()
                    else:
                        reserve[0] = 10
                    if c > 0:
                        enqueue_y(c - 1)
                    for h in range(GROUP):
                        chain = attn_head(c, h, chain)
            else:
                for c in range(NCH):
                    proj_pass(c, ["k", "v", 0, 1], {})
                    vtrans(c)
                    proj_pass(c, [2, 3], {})
                for c in range(NCH):
                    if mode == "generic":
                        load_masks(c)
                    if c > 0:
                        enqueue_y(c - 1)
                    for h in range(GROUP):
                        chain = attn_head(c, h, chain)
            reserve[0] = 0
            chain[0]()
            chain[1]()
            drain_fill()
            for it in range(CH // 128):
                cell = {}
                for nh in range(NCH):
                    for h in range(GROUP):
                        make_yh(NCH - 1, it, nh, h, cell)()
    nc.finalize()
    return nc


def _get_prog(mode):
    if mode not in _PROG_CACHE:
        _PROG_CACHE[mode] = _build(mode)
    return _PROG_CACHE[mode]


def kernel(x, mask, wq, bq, wk, bk, wv, bv, wo, bo):
    global LAST_EXEC_NS, LAST_RESULTS
    from concourse.bass_utils import run_bass_kernel_spmd

    x = np.asarray(x, dtype=np.float32)
    mask = np.asarray(mask)
    wq = np.asarray(wq, dtype=np.float32)
    bq = np.asarray(bq, dtype=np.float32)
    wk = np.asarray(wk, dtype=np.float32)
    bk = np.asarray(bk, dtype=np.float32)
    wv = np.asarray(wv, dtype=np.float32)
    bv = np.asarray(bv, dtype=np.float32)
    wo = np.asarray(wo, dtype=np.float32)
    bo = np.asarray(bo, dtype=np.float32)

    m2 = mask[0, 0]
    if np.array_equal(m2 != 0, np.tril(np.ones((S, S), dtype=bool))):
        mode = "causal"
    elif np.all(m2 != 0):
        mode = "full"
    else:
        mode = "generic"

    f16 = np.float16
    xT = np.ascontiguousarray(x[0].T).astype(f16)
    xB = np.ascontiguousarray(xT[:, CH:])
    if mode == "causal":
        mstrip = np.triu(np.ones((128, 128), dtype=f16))
    in_maps = []
    for c in range(NCORES):
        qs = slice(4 * c * 128, (4 * c + 4) * 128)
        ks = slice(c * 128, (c + 1) * 128)
        biasp = np.zeros((128, 6), np.float32)
        biasp[:, 0:4] = (bq[qs] * INV_SQRT_D).reshape(4, 128).T
        biasp[:, 4] = bk[ks]
        biasp[:, 5] = bv[ks]
        pk = np.concatenate(
            [wq[:, qs].astype(f16), wk[:, ks].astype(f16),
             wv[:, ks].astype(f16), xT[:, 0:CH]], axis=1)
        im = {
            "pk": np.ascontiguousarray(pk),
            "xB": xB,
            "wo": np.ascontiguousarray(wo[qs, :]).astype(f16),
            "biasp": biasp,
        }
        if mode == "causal":
            im["mstrip"] = mstrip
        if mode == "generic":
            im["maskT"] = np.ascontiguousarray((m2 != 0).T).astype(f16)
        in_maps.append(im)

    nc = _get_prog(mode)
    res = run_bass_kernel_spmd(nc, in_maps, list(range(NCORES)), trace=TRACE)
    LAST_EXEC_NS = res.exec_time_ns
    LAST_RESULTS = res
    y = np.zeros((S, HIDDEN), np.float64)
    for r in res.results:
        y += r["y"].astype(np.float64)
    y = (y + bo.astype(np.float64)).astype(np.float32)
    return y[None]


# revision 21
# speedup vs baseline: 1.0791x; 1.0791x over previous
"""GQA attention layer for Trainium2, tensor-parallel over kv-heads on 8 NeuronCores.

Problem: x:(1,2048,2048) f32, causal mask; q/k/v/o projections with
NUM_HEADS=32, NUM_KV_HEADS=8, HEAD_DIM=128, GROUP=4.

Sharding: core c owns kv-head c and its 4 query heads (columns 4c*128..(4c+4)*128
of wq, rows of wo). Each core computes a partial y_c = attnout_c @ wo_c; the host
sums the 8 partials and adds bo.

Dataflow on each core (all "transposed" so no on-chip transposes of the big
probability matrix are ever needed):
  qT[d,i] = wq_c.T(h-major) accumulation:  matmul(lhsT=wq_kt, rhs=xT_kt)
  kT[d,j], vT[d,j] likewise;  v[j,d] via 16 PE transposes of vT.
  sT[j,i] = matmul(lhsT=kT_jtile, rhs=qT_chunk)   (contraction = head_dim, 1 mm)
  e = exp(sT) on ACT (1/sqrt(d) folded into the qT drain), causal-masked by a
      [128,128] triu multiply on the triangular quarter of diagonal j-tiles.
  colsum: e-tiles are pre-summed on DVE (fp16, 2x rate) into acc; ONE
      PE matmul ones.T @ acc per (h,chunk) yields the softmax denominator
      (the baseline burned one PE matmul per block on this).
  recip = 1/colsum (DVE); broadcast to 128 partitions with a k=1 PE matmul;
  aoT = avpsum * recip (DVE), fp16
  y[i,hid] += aoT_head_tile.T @ wo_head  (4 head k-tiles), fp16 out, DMA to HBM;
  the host sums the 8 fp16 partials in f64 and adds bo.

Causality: for i-chunk c (512 wide) only j-tiles 0..4c+3 are computed, and the
above-diagonal quarters of diagonal j-tiles are skipped (ragged score/av
matmuls over qT[:, i0:]).

Scheduling: the PE issues in order, and every stall resets its p-state ramp
(full 2.4 GHz only after ~3us of continuous execution), so stall-free emission
order matters more than instruction count.  The emitter weaves independent
work into the attention stream via a FIFO fill queue: y-projection tiles of
chunk c-1 and the cs/recip-broadcast chain of the previous head are emitted
between score/av matmuls, and next-chunk projections run right after each
attention chunk.  x is DMA'd in per-chunk pieces so chunk-0 projections start
~2us in instead of waiting for the full x load.
"""

import math
from collections import deque

import numpy as np

HIDDEN = 2048
HEAD_DIM = 128
NUM_HEADS = 32
NUM_KV = 8
GROUP = NUM_HEADS // NUM_KV
S = 2048
NCORES = 8
CH = 512                      # i-chunk width
NCH = S // CH                 # 4 i-chunks
KT = HIDDEN // 128            # 16 contraction tiles over hidden
NJT = S // 128                # 16 j-tiles
INV_SQRT_D = 1.0 / math.sqrt(HEAD_DIM)
LAG = 4                       # score->av lag in j-blocks

# Module-level knobs for test.py (the grading harness uses the defaults).
TRACE = False
LAST_EXEC_NS = None
LAST_RESULTS = None

_PROG_CACHE = {}


def _build(mode):
    """mode: 'causal' (skip upper blocks + ragged diagonals), 'full' (all-ones
    mask), 'generic' (multiplicative fp16 mask tiles from HBM)."""
    import concourse.bacc as bacc
    import concourse.tile as tile
    import concourse.mybir as mybir
    from concourse.masks import make_identity

    f32 = mybir.dt.float32
    f16 = mybir.dt.float16
    Ident = mybir.ActivationFunctionType.Identity
    Exp = mybir.ActivationFunctionType.Exp

    nc = bacc.Bacc(None, target_bir_lowering=False)

    # PK packs, per hidden k-tile, [wq(512) | wk(128) | wv(128) | xA(512)] so
    # chunk-0 projections are fed by ONE dma per k-tile (issue-rate bound
    # otherwise: ~617ns per descriptor on the Sync queue).
    PKW = GROUP * HEAD_DIM + 2 * HEAD_DIM + CH
    pk_d = nc.dram_tensor("pk", [HIDDEN, PKW], f16, kind="ExternalInput")
    xB_d = nc.dram_tensor("xB", [HIDDEN, S - CH], f16, kind="ExternalInput")
    wo_d = nc.dram_tensor("wo", [GROUP * HEAD_DIM, HIDDEN], f16, kind="ExternalInput")
    bias_d = nc.dram_tensor("biasp", [128, 6], f32, kind="ExternalInput")
    if mode == "causal":
        ms_d = nc.dram_tensor("mstrip", [128, 128], f16, kind="ExternalInput")
    if mode == "generic":
        mk_d = nc.dram_tensor("maskT", [S, S], f16, kind="ExternalInput")
    y_d = nc.dram_tensor("y", [S, HIDDEN], f16, kind="ExternalOutput")

    def nblocks(c):
        return 4 * c + 4 if mode == "causal" else NJT

    with tile.TileContext(nc) as tc:
        with (
            tc.tile_pool(name="consts", bufs=1) as consts,
            tc.tile_pool(name="xw", bufs=1) as xw,
            tc.tile_pool(name="proj", bufs=1) as proj,
            tc.tile_pool(name="epool", bufs=6) as epool,
            tc.tile_pool(name="accp", bufs=1) as accp,
            tc.tile_pool(name="rpool", bufs=1) as rpool,
            tc.tile_pool(name="ypool", bufs=6) as ypool,
            tc.tile_pool(name="pp", bufs=4, space="PSUM") as pp,
            tc.tile_pool(name="spp", bufs=2, space="PSUM") as spp,
            tc.tile_pool(name="avp", bufs=1, space="PSUM") as avp,
            tc.tile_pool(name="nrm", bufs=1, space="PSUM") as nrm,
        ):
            # ---- constants ----
            ident = consts.tile([128, 128], f16, tag="ident", name="ident")
            make_identity(nc, ident)
            ones_col = consts.tile([128, 1], f16, tag="ones_col", name="ones_col")
            nc.vector.memset(ones_col, 1.0)
            # ---- input loads (all on Sync, priority order; y writebacks go
            # to the otherwise-idle GpSimd queue).  bias/mstrip are loaded
            # after the PK tiles: they are first needed ~8us later than pk0,
            # which gates the very first matmul. ----
            pk_sb, xB, wo_sb = [], [], []
            bias_sb = mstrip = None
            for kt in range(KT):
                pkt = xw.tile([128, PKW], f16, tag=f"pk{kt}", name=f"pk{kt}")
                nc.sync.dma_start(out=pkt, in_=pk_d[kt * 128:(kt + 1) * 128, :])
                pk_sb.append(pkt)
                if kt == 3:
                    bias_sb = consts.tile([128, 6], f32, tag="bias",
                                          name="bias_sb")
                    nc.sync.dma_start(out=bias_sb, in_=bias_d[:, :])
                    if mode == "causal":
                        mstrip = consts.tile([128, 128], f16, tag="mstrip",
                                             name="mstrip")
                        nc.sync.dma_start(out=mstrip, in_=ms_d[:, :])
            for kt in range(KT):
                xb = xw.tile([128, S - CH], f16, tag=f"xB{kt}", name=f"xB{kt}")
                nc.sync.dma_start(out=xb, in_=xB_d[kt * 128:(kt + 1) * 128, :])
                xB.append(xb)
            for h in range(GROUP):
                wot = xw.tile([128, HIDDEN], f16, tag=f"wo{h}", name=f"wo{h}")
                nc.sync.dma_start(out=wot, in_=wo_d[h * 128:(h + 1) * 128, :])
                wo_sb.append(wot)

            def xr(c, kt):
                if c == 0:
                    return pk_sb[kt][:, 768:768 + CH]
                return xB[kt][:, (c - 1) * CH:c * CH]

            # ---- fill queue: independent PE work woven into stall-prone
            # stretches of the stream.  `reserve` holds back items so the
            # last chunk's softmax-chain latency can be covered at the very
            # end (no next chunk to weave into). ----
            fill = deque()
            reserve = [0]

            def pump(n=1):
                for _ in range(min(n, len(fill) - reserve[0])):
                    fill.popleft()()

            def drain_fill():
                while fill:
                    fill.popleft()()

            kT_c = [None] * NCH
            qT = {}
            v_sb = [None] * NJT
            vT_cur = [None]
            aoT = {}
            mask_sb = {}

            # ---- projections for chunk c (two passes, <=4 PSUM banks) ----
            def proj_pass(c, spec, pumps, fill_rate=0):
                """spec: list of ('k'|'v'|0..3).  pumps: dict kt -> closure."""
                pss = [pp.tile([128, CH], f32, tag="pp", name=f"P{c}{s}")
                       for s in spec]
                for kt in range(KT):
                    for ps, s in zip(pss, spec):
                        if s == "k":
                            lhsT = pk_sb[kt][:, 512:640]
                        elif s == "v":
                            lhsT = pk_sb[kt][:, 640:768]
                        else:
                            lhsT = pk_sb[kt][:, s * 128:(s + 1) * 128]
                        nc.tensor.matmul(ps, lhsT=lhsT, rhs=xr(c, kt),
                                         start=(kt == 0), stop=(kt == KT - 1))
                    if kt in pumps:
                        pumps[kt]()
                    elif fill_rate:
                        pump(fill_rate)
                for ps, s in zip(pss, spec):
                    if s == "k":
                        kt_t = proj.tile([128, CH], f16, tag=f"kT{c}",
                                         name=f"kT{c}")
                        nc.scalar.activation(kt_t, ps, Ident, bias=bias_sb[:, 4:5])
                        kT_c[c] = kt_t
                    elif s == "v":
                        vt_t = proj.tile([128, CH], f16, tag=f"vT{c % 2}",
                                         name=f"vT{c}")
                        nc.scalar.activation(vt_t, ps, Ident, bias=bias_sb[:, 5:6])
                        vT_cur[0] = vt_t
                    else:
                        qt_t = proj.tile([128, CH], f16, tag=f"q{s}_{c % 2}",
                                         name=f"q{s}_{c}")
                        nc.scalar.activation(qt_t, ps, Ident,
                                             bias=bias_sb[:, s:s + 1],
                                             scale=INV_SQRT_D)
                        qT[(s, c)] = qt_t

            def vtrans(c):
                for d in range(4):
                    b = 4 * c + d
                    tp = spp.tile([128, 128], f16, tag="s", name=f"tp{b}")
                    nc.tensor.transpose(tp, vT_cur[0][:, d * 128:(d + 1) * 128],
                                        ident)
                    vt = proj.tile([128, 128], f16, tag=f"v{b}", name=f"v{b}")
                    nc.vector.tensor_copy(vt, tp)
                    v_sb[b] = vt

            # ---- attention for one (chunk, head) ----
            def attn_head(c, h, chain):
                nb = nblocks(c)
                qt = qT[(h, c)]
                par = (h + c) % 2
                acc = accp.tile([128, CH], f16, tag=f"acc{par}",
                                name=f"acc{h}_{c}")
                av = avp.tile([128, CH], f32, tag="av", name=f"av{h}_{c}")
                pend = deque()

                def emit_av():
                    b, i0, e = pend.popleft()
                    nc.tensor.matmul(av[:, i0:], lhsT=v_sb[b], rhs=e[:, i0:],
                                     start=(b == 0), stop=(b == nb - 1),
                                     skip_group_check=True)

                for b in range(nb):
                    diag = mode == "causal" and b >= 4 * c
                    i0 = 128 * (b - 4 * c) if diag else 0
                    sp = spp.tile([128, CH], f32, tag="s", name=f"s{h}_{c}_{b}")
                    nc.tensor.matmul(sp[:, i0:],
                                     lhsT=kT_c[b // 4][:, (b % 4) * 128:
                                                       (b % 4 + 1) * 128],
                                     rhs=qt[:, i0:], start=True, stop=True)
                    e = epool.tile([128, CH], f16, tag="e", name=f"e{h}_{c}_{b}")
                    nc.scalar.activation(e[:, i0:], sp[:, i0:], Exp)
                    if diag:
                        nc.vector.tensor_mul(e[:, i0:i0 + 128],
                                             e[:, i0:i0 + 128], mstrip)
                    elif mode == "generic":
                        nc.vector.tensor_mul(e, e, mask_sb[b])
                    if b == 0:
                        nc.vector.tensor_copy(acc, e)
                    else:
                        nc.vector.tensor_add(acc[:, i0:], acc[:, i0:],
                                             e[:, i0:])
                    pend.append((b, i0, e))
                    if chain is not None and b == 0:
                        chain[0]()
                    elif chain is not None and b == 2:
                        chain[1]()
                        pump(1)
                    else:
                        pump(1)
                    if len(pend) > LAG:
                        emit_av()
                while pend:
                    emit_av()

                cell = {}

                def cs_fn():
                    cs = nrm.tile([1, CH], f32, tag="n", name=f"cs{h}_{c}")
                    nc.tensor.matmul(cs, lhsT=ones_col, rhs=acc,
                                     start=True, stop=True)
                    rc = rpool.tile([1, CH], f32, tag=f"rc{par}",
                                    name=f"rc{h}_{c}")
                    nc.vector.reciprocal_approx_fast(rc, cs)
                    cell["rc"] = rc

                def rb_fn():
                    # partition-broadcast the reciprocal off the PE (the
                    # baseline burned a k=1 matmul + drain copy on this)
                    rb = rpool.tile([128, CH], f32, tag=f"rb{par}",
                                    name=f"rb{h}_{c}")
                    nc.gpsimd.partition_broadcast(rb, cell["rc"], 128)
                    ao = proj.tile([128, CH], f16, tag=f"ao{h}_{c % 2}",
                                   name=f"ao{h}_{c}")
                    nc.vector.tensor_mul(ao, av, rb)
                    aoT[(h, c)] = ao

                return (cs_fn, rb_fn)

            # ---- y projection fill items (one per (it, nh, h) matmul so the
            # A-phase can pump exactly one PE op per attention block).
            # Writebacks: chunks 0-2 batch 4 column tiles into one [128,2048]
            # DMA per row-tile; the last chunk uses per-column DMAs to keep
            # the tail short.  All y DMAs ride the GpSimd queue. ----
            def make_yh(c, it, nh, h, cell):
                def f():
                    if h == 0:
                        cell["yp"] = pp.tile([128, CH], f32, tag="pp",
                                             name=f"yp{c}_{it}_{nh}")
                    yp = cell["yp"]
                    nc.tensor.matmul(
                        yp, lhsT=aoT[(h, c)][:, it * 128:(it + 1) * 128],
                        rhs=wo_sb[h][:, nh * CH:(nh + 1) * CH],
                        start=(h == 0), stop=(h == GROUP - 1))
                    if h != GROUP - 1:
                        return
                    r0 = c * CH + it * 128
                    if c == NCH - 1:
                        ysb = ypool.tile([128, CH], f16, tag="y",
                                         name=f"y{c}_{it}_{nh}")
                        nc.scalar.copy(ysb, yp)
                        nc.gpsimd.dma_start(
                            out=y_d[r0:r0 + 128, nh * CH:(nh + 1) * CH],
                            in_=ysb)
                    else:
                        if nh == 0:
                            cell["ysb"] = ypool.tile([128, HIDDEN], f16,
                                                     tag="yb",
                                                     name=f"yb{c}_{it}")
                        ysb = cell["ysb"]
                        nc.vector.tensor_copy(
                            ysb[:, nh * CH:(nh + 1) * CH], yp)
                        if nh == NCH - 1:
                            nc.gpsimd.dma_start(out=y_d[r0:r0 + 128, :],
                                                in_=ysb)
                return f

            def enqueue_y(c):
                for it in range(CH // 128):
                    cell = {}
                    for nh in range(NCH):
                        for h in range(GROUP):
                            fill.append(make_yh(c, it, nh, h, cell))

            def load_masks(c):
                for b in range(nblocks(c)):
                    mt = proj.tile([128, CH], f16, tag=f"m{b}", name=f"m{b}_{c}")
                    nc.sync.dma_start(
                        out=mt,
                        in_=mk_d[b * 128:(b + 1) * 128, c * CH:(c + 1) * CH])
                    mask_sb[b] = mt

            # ---- top-level emission ----
            chain = None
            if mode == "causal":
                for c in range(NCH):
                    p1 = {2: chain[0]} if chain is not None else {}
                    p2 = {0: chain[1]} if chain is not None else {}
                    proj_pass(c, ["k", "v", 0, 1], p1)
                    vtrans(c)
                    proj_pass(c, [2, 3], p2, fill_rate=1)
                    chain = None
                    if c < NCH - 1:
                        drain_fill()
                    if c > 0:
                        enqueue_y(c - 1)
                    for h in range(GROUP):
                        chain = attn_head(c, h, chain)
            else:
                for c in range(NCH):
                    proj_pass(c, ["k", "v", 0, 1], {})
                    vtrans(c)
                    proj_pass(c, [2, 3], {})
                for c in range(NCH):
                    if mode == "generic":
                        load_masks(c)
                    if c > 0:
                        enqueue_y(c - 1)
                    for h in range(GROUP):
                        chain = attn_head(c, h, chain)
            pump(2)
            chain[0]()
            chain[1]()
            drain_fill()
            # Last chunk's y tiles: open the first four psum groups with the
            # h3 matmul deferred, so the PE has ~3us of ready work while the
            # last head's colsum->recip->broadcast->normalize chain drains.
            open_cells = [{} for _ in range(NCH)]
            for nh in range(NCH):
                for h in range(GROUP - 1):
                    make_yh(NCH - 1, 0, nh, h, open_cells[nh])()
            for nh in range(NCH):
                make_yh(NCH - 1, 0, nh, GROUP - 1, open_cells[nh])()
            for it in range(1, CH // 128):
                cell = {}
                for nh in range(NCH):
                    for h in range(GROUP):
                        make_yh(NCH - 1, it, nh, h, cell)()
    nc.finalize()
    return nc


def _get_prog(mode):
    if mode not in _PROG_CACHE:
        _PROG_CACHE[mode] = _build(mode)
    return _PROG_CACHE[mode]


def kernel(x, mask, wq, bq, wk, bk, wv, bv, wo, bo):
    global LAST_EXEC_NS, LAST_RESULTS
    from concourse.bass_utils import run_bass_kernel_spmd

    x = np.asarray(x, dtype=np.float32)
    mask = np.asarray(mask)
    wq = np.asarray(wq, dtype=np.float32)
    bq = np.asarray(bq, dtype=np.float32)
    wk = np.asarray(wk, dtype=np.float32)
    bk = np.asarray(bk, dtype=np.float32)
    wv = np.asarray(wv, dtype=np.float32)
    bv = np.asarray(bv, dtype=np.float32)
    wo = np.asarray(wo, dtype=np.float32)
    bo = np.asarray(bo, dtype=np.float32)

    m2 = mask[0, 0]
    if np.array_equal(m2 != 0, np.tril(np.ones((S, S), dtype=bool))):
        mode = "causal"
    elif np.all(m2 != 0):
        mode = "full"
    else:
        mode = "generic"

    f16 = np.float16
    xT = np.ascontiguousarray(x[0].T).astype(f16)
    xB = np.ascontiguousarray(xT[:, CH:])
    if mode == "causal":
        mstrip = np.triu(np.ones((128, 128), dtype=f16))
    in_maps = []
    for c in range(NCORES):
        qs = slice(4 * c * 128, (4 * c + 4) * 128)
        ks = slice(c * 128, (c + 1) * 128)
        biasp = np.zeros((128, 6), np.float32)
        biasp[:, 0:4] = (bq[qs] * INV_SQRT_D).reshape(4, 128).T
        biasp[:, 4] = bk[ks]
        biasp[:, 5] = bv[ks]
        pk = np.concatenate(
            [wq[:, qs].astype(f16), wk[:, ks].astype(f16),
             wv[:, ks].astype(f16), xT[:, 0:CH]], axis=1)
        im = {
            "pk": np.ascontiguousarray(pk),
            "xB": xB,
            "wo": np.ascontiguousarray(wo[qs, :]).astype(f16),
            "biasp": biasp,
        }
        if mode == "causal":
            im["mstrip"] = mstrip
        if mode == "generic":
            im["maskT"] = np.ascontiguousarray((m2 != 0).T).astype(f16)
        in_maps.append(im)

    nc = _get_prog(mode)
    res = run_bass_kernel_spmd(nc, in_maps, list(range(NCORES)), trace=TRACE)
    LAST_EXEC_NS = res.exec_time_ns
    LAST_RESULTS = res
    y = np.zeros((S, HIDDEN), np.float64)
    for r in res.results:
        y += r["y"].astype(np.float64)
    y = (y + bo.astype(np.float64)).astype(np.float32)
    return y[None]


# revision 22
# speedup vs baseline: 1.1837x; 1.0970x over previous
"""GQA attention layer for Trainium2, tensor-parallel over kv-heads on 8 NeuronCores.

Problem: x:(1,2048,2048) f32, causal mask; q/k/v/o projections with
NUM_HEADS=32, NUM_KV_HEADS=8, HEAD_DIM=128, GROUP=4.

Sharding: core c owns kv-head c and its 4 query heads (columns 4c*128..(4c+4)*128
of wq, rows of wo). Each core computes a partial y_c = attnout_c @ wo_c; the host
sums the 8 partials and adds bo.

Dataflow on each core (all "transposed" so no on-chip transposes of the big
probability matrix are ever needed):
  qT[d,i] = wq_c.T(h-major) accumulation:  matmul(lhsT=wq_kt, rhs=xT_kt)
  kT[d,j], vT[d,j] likewise;  v[j,d] via 16 PE transposes of vT.
  sT[j,i] = matmul(lhsT=kT_jtile, rhs=qT_chunk)   (contraction = head_dim, 1 mm)
  e = exp(sT) on ACT (1/sqrt(d) folded into the qT drain), causal-masked by a
      [128,128] triu multiply on the triangular quarter of diagonal j-tiles.
  colsum: e-tiles are pre-summed on DVE (fp16, 2x rate) into acc; ONE
      PE matmul ones.T @ acc per (h,chunk) yields the softmax denominator
      (the baseline burned one PE matmul per block on this).
  recip = 1/colsum (DVE); broadcast to 128 partitions with a k=1 PE matmul;
  aoT = avpsum * recip (DVE), fp16
  y[i,hid] += aoT_head_tile.T @ wo_head  (4 head k-tiles), fp16 out, DMA to HBM;
  the host sums the 8 fp16 partials in f64 and adds bo.

Causality: for i-chunk c (512 wide) only j-tiles 0..4c+3 are computed, and the
above-diagonal quarters of diagonal j-tiles are skipped (ragged score/av
matmuls over qT[:, i0:]).

Scheduling: the PE issues in order, and every stall resets its p-state ramp
(full 2.4 GHz only after ~3us of continuous execution), so stall-free emission
order matters more than instruction count.  The emitter weaves independent
work into the attention stream via a FIFO fill queue: y-projection tiles of
chunk c-1 and the cs/recip-broadcast chain of the previous head are emitted
between score/av matmuls, and next-chunk projections run right after each
attention chunk.  x is DMA'd in per-chunk pieces so chunk-0 projections start
~2us in instead of waiting for the full x load.
"""

import math
from collections import deque

import numpy as np

HIDDEN = 2048
HEAD_DIM = 128
NUM_HEADS = 32
NUM_KV = 8
GROUP = NUM_HEADS // NUM_KV
S = 2048
NCORES = 8
CH = 512                      # i-chunk width
NCH = S // CH                 # 4 i-chunks
KT = HIDDEN // 128            # 16 contraction tiles over hidden
NJT = S // 128                # 16 j-tiles
INV_SQRT_D = 1.0 / math.sqrt(HEAD_DIM)
LAG = 4                       # score->av lag in j-blocks

# Module-level knobs for test.py (the grading harness uses the defaults).
TRACE = False
LAST_EXEC_NS = None
LAST_RESULTS = None

_PROG_CACHE = {}


def _build(mode):
    """mode: 'causal' (skip upper blocks + ragged diagonals), 'full' (all-ones
    mask), 'generic' (multiplicative fp16 mask tiles from HBM)."""
    import concourse.bacc as bacc
    import concourse.tile as tile
    import concourse.mybir as mybir
    from concourse.masks import make_identity

    f32 = mybir.dt.float32
    f16 = mybir.dt.float16
    Ident = mybir.ActivationFunctionType.Identity
    Exp = mybir.ActivationFunctionType.Exp

    nc = bacc.Bacc(None, target_bir_lowering=False)

    # PK packs, per hidden k-tile, [wq(512) | wk(128) | wv(128) | xA(512)] so
    # chunk-0 projections are fed by ONE dma per k-tile (issue-rate bound
    # otherwise: ~617ns per descriptor on the Sync queue).
    PKW = GROUP * HEAD_DIM + 2 * HEAD_DIM + CH
    pk_d = nc.dram_tensor("pk", [HIDDEN, PKW], f16, kind="ExternalInput")
    xB_d = nc.dram_tensor("xB", [HIDDEN, S - CH], f16, kind="ExternalInput")
    wo_d = nc.dram_tensor("wo", [GROUP * HEAD_DIM, HIDDEN], f16, kind="ExternalInput")
    bias_d = nc.dram_tensor("biasp", [128, 6], f32, kind="ExternalInput")
    if mode == "causal":
        ms_d = nc.dram_tensor("mstrip", [128, 128], f16, kind="ExternalInput")
    if mode == "generic":
        mk_d = nc.dram_tensor("maskT", [S, S], f16, kind="ExternalInput")
    y_d = nc.dram_tensor("y", [S, HIDDEN], f16, kind="ExternalOutput")

    def nblocks(c):
        return 4 * c + 4 if mode == "causal" else NJT

    with tile.TileContext(nc) as tc:
        with (
            tc.tile_pool(name="consts", bufs=1) as consts,
            tc.tile_pool(name="xw", bufs=1) as xw,
            tc.tile_pool(name="proj", bufs=1) as proj,
            tc.tile_pool(name="epool", bufs=6) as epool,
            tc.tile_pool(name="accp", bufs=1) as accp,
            tc.tile_pool(name="rpool", bufs=1) as rpool,
            tc.tile_pool(name="ypool", bufs=6) as ypool,
            tc.tile_pool(name="pp", bufs=4, space="PSUM") as pp,
            tc.tile_pool(name="spp", bufs=2, space="PSUM") as spp,
            tc.tile_pool(name="avp", bufs=1, space="PSUM") as avp,
            tc.tile_pool(name="nrm", bufs=1, space="PSUM") as nrm,
        ):
            # ---- constants ----
            ident = consts.tile([128, 128], f16, tag="ident", name="ident")
            make_identity(nc, ident)
            ones_col = consts.tile([128, 1], f16, tag="ones_col", name="ones_col")
            nc.vector.memset(ones_col, 1.0)
            # ---- input loads (all on Sync, priority order; y writebacks go
            # to the otherwise-idle GpSimd queue).  bias/mstrip are loaded
            # after the PK tiles: they are first needed ~8us later than pk0,
            # which gates the very first matmul. ----
            pk_sb, xB, wo_sb = [], [], []
            bias_sb = mstrip = None
            for kt in range(KT):
                pkt = xw.tile([128, PKW], f16, tag=f"pk{kt}", name=f"pk{kt}")
                nc.sync.dma_start(out=pkt, in_=pk_d[kt * 128:(kt + 1) * 128, :])
                pk_sb.append(pkt)
                if kt == 3:
                    bias_sb = consts.tile([128, 6], f32, tag="bias",
                                          name="bias_sb")
                    nc.sync.dma_start(out=bias_sb, in_=bias_d[:, :])
                    if mode == "causal":
                        mstrip = consts.tile([128, 128], f16, tag="mstrip",
                                             name="mstrip")
                        nc.sync.dma_start(out=mstrip, in_=ms_d[:, :])
            for kt in range(KT):
                xb = xw.tile([128, S - CH], f16, tag=f"xB{kt}", name=f"xB{kt}")
                nc.sync.dma_start(out=xb, in_=xB_d[kt * 128:(kt + 1) * 128, :])
                xB.append(xb)
            for h in range(GROUP):
                wot = xw.tile([128, HIDDEN], f16, tag=f"wo{h}", name=f"wo{h}")
                nc.sync.dma_start(out=wot, in_=wo_d[h * 128:(h + 1) * 128, :])
                wo_sb.append(wot)

            def xr(c, kt):
                if c == 0:
                    return pk_sb[kt][:, 768:768 + CH]
                return xB[kt][:, (c - 1) * CH:c * CH]

            # ---- fill queue: independent PE work woven into stall-prone
            # stretches of the stream.  `reserve` holds back items so the
            # last chunk's softmax-chain latency can be covered at the very
            # end (no next chunk to weave into). ----
            fill = deque()
            reserve = [0]

            def pump(n=1):
                for _ in range(min(n, len(fill) - reserve[0])):
                    fill.popleft()()

            def drain_fill():
                while fill:
                    fill.popleft()()

            kT_c = [None] * NCH
            qT = {}
            v_sb = [None] * NJT
            vT_cur = [None]
            aoT = {}
            mask_sb = {}

            # ---- projections for chunk c (two passes, <=4 PSUM banks) ----
            def proj_pass(c, spec, pumps, fill_rate=0):
                """spec: list of ('k'|'v'|0..3).  pumps: dict kt -> closure."""
                pss = [pp.tile([128, CH], f32, tag="pp", name=f"P{c}{s}")
                       for s in spec]
                for kt in range(KT):
                    for ps, s in zip(pss, spec):
                        if s == "k":
                            lhsT = pk_sb[kt][:, 512:640]
                        elif s == "v":
                            lhsT = pk_sb[kt][:, 640:768]
                        else:
                            lhsT = pk_sb[kt][:, s * 128:(s + 1) * 128]
                        nc.tensor.matmul(ps, lhsT=lhsT, rhs=xr(c, kt),
                                         start=(kt == 0), stop=(kt == KT - 1))
                    if kt in pumps:
                        pumps[kt]()
                    elif fill_rate:
                        pump(fill_rate)
                for ps, s in zip(pss, spec):
                    if s == "k":
                        kt_t = proj.tile([128, CH], f16, tag=f"kT{c}",
                                         name=f"kT{c}")
                        nc.scalar.activation(kt_t, ps, Ident, bias=bias_sb[:, 4:5])
                        kT_c[c] = kt_t
                    elif s == "v":
                        vt_t = proj.tile([128, CH], f16, tag=f"vT{c % 2}",
                                         name=f"vT{c}")
                        nc.scalar.activation(vt_t, ps, Ident, bias=bias_sb[:, 5:6])
                        vT_cur[0] = vt_t
                    else:
                        # causal runs P(c) between attention chunks, so two
                        # rotating buffers suffice; the fallback modes run all
                        # projections upfront and need per-chunk tiles.
                        qtag = c % 2 if mode == "causal" else c
                        qt_t = proj.tile([128, CH], f16, tag=f"q{s}_{qtag}",
                                         name=f"q{s}_{c}")
                        nc.scalar.activation(qt_t, ps, Ident,
                                             bias=bias_sb[:, s:s + 1],
                                             scale=INV_SQRT_D)
                        qT[(s, c)] = qt_t

            def vtrans(c):
                for d in range(4):
                    b = 4 * c + d
                    tp = spp.tile([128, 128], f16, tag="s", name=f"tp{b}")
                    nc.tensor.transpose(tp, vT_cur[0][:, d * 128:(d + 1) * 128],
                                        ident)
                    vt = proj.tile([128, 128], f16, tag=f"v{b}", name=f"v{b}")
                    nc.vector.tensor_copy(vt, tp)
                    v_sb[b] = vt

            # ---- attention for one (chunk, head) ----
            def attn_head(c, h, chain):
                nb = nblocks(c)
                qt = qT[(h, c)]
                par = (h + c) % 2
                acc = accp.tile([128, CH], f16, tag=f"acc{par}",
                                name=f"acc{h}_{c}")
                av = avp.tile([128, CH], f32, tag="av", name=f"av{h}_{c}")
                pend = deque()

                def emit_av():
                    b, i0, e = pend.popleft()
                    nc.tensor.matmul(av[:, i0:], lhsT=v_sb[b], rhs=e[:, i0:],
                                     start=(b == 0), stop=(b == nb - 1),
                                     skip_group_check=True)

                for b in range(nb):
                    diag = mode == "causal" and b >= 4 * c
                    i0 = 128 * (b - 4 * c) if diag else 0
                    sp = spp.tile([128, CH], f32, tag="s", name=f"s{h}_{c}_{b}")
                    nc.tensor.matmul(sp[:, i0:],
                                     lhsT=kT_c[b // 4][:, (b % 4) * 128:
                                                       (b % 4 + 1) * 128],
                                     rhs=qt[:, i0:], start=True, stop=True)
                    e = epool.tile([128, CH], f16, tag="e", name=f"e{h}_{c}_{b}")
                    nc.scalar.activation(e[:, i0:], sp[:, i0:], Exp)
                    if diag:
                        nc.vector.tensor_mul(e[:, i0:i0 + 128],
                                             e[:, i0:i0 + 128], mstrip)
                    elif mode == "generic":
                        nc.vector.tensor_mul(e, e, mask_sb[b])
                    if b == 0:
                        nc.vector.tensor_copy(acc, e)
                    else:
                        nc.vector.tensor_add(acc[:, i0:], acc[:, i0:],
                                             e[:, i0:])
                    pend.append((b, i0, e))
                    if chain is not None and b == 0:
                        chain[0]()
                    elif chain is not None and b == 2:
                        chain[1]()
                        pump(1)
                    else:
                        pump(1)
                    if len(pend) > LAG:
                        emit_av()
                while pend:
                    emit_av()

                cell = {}

                def cs_fn():
                    cs = nrm.tile([1, CH], f32, tag="n", name=f"cs{h}_{c}")
                    nc.tensor.matmul(cs, lhsT=ones_col, rhs=acc,
                                     start=True, stop=True)
                    rc = rpool.tile([1, CH], f32, tag=f"rc{par}",
                                    name=f"rc{h}_{c}")
                    nc.vector.reciprocal_approx_fast(rc, cs)
                    cell["rc"] = rc

                def rb_fn():
                    # partition-broadcast the reciprocal off the PE (the
                    # baseline burned a k=1 matmul + drain copy on this)
                    rb = rpool.tile([128, CH], f32, tag=f"rb{par}",
                                    name=f"rb{h}_{c}")
                    nc.gpsimd.partition_broadcast(rb, cell["rc"], 128)
                    ao = proj.tile([128, CH], f16, tag=f"ao{h}_{c % 2}",
                                   name=f"ao{h}_{c}")
                    nc.vector.tensor_mul(ao, av, rb)
                    aoT[(h, c)] = ao

                return (cs_fn, rb_fn)

            # ---- y projection fill items (one per (it, nh, h) matmul so the
            # A-phase can pump exactly one PE op per attention block).
            # Writebacks: chunks 0-2 batch 4 column tiles into one [128,2048]
            # DMA per row-tile; the last chunk uses per-column DMAs to keep
            # the tail short.  All y DMAs ride the GpSimd queue. ----
            def make_yh(c, it, nh, h, cell):
                def f():
                    if h == 0:
                        cell["yp"] = pp.tile([128, CH], f32, tag="pp",
                                             name=f"yp{c}_{it}_{nh}")
                    yp = cell["yp"]
                    nc.tensor.matmul(
                        yp, lhsT=aoT[(h, c)][:, it * 128:(it + 1) * 128],
                        rhs=wo_sb[h][:, nh * CH:(nh + 1) * CH],
                        start=(h == 0), stop=(h == GROUP - 1))
                    if h != GROUP - 1:
                        return
                    r0 = c * CH + it * 128
                    if c == NCH - 1:
                        ysb = ypool.tile([128, CH], f16, tag="y",
                                         name=f"y{c}_{it}_{nh}")
                        nc.scalar.copy(ysb, yp)
                        nc.gpsimd.dma_start(
                            out=y_d[r0:r0 + 128, nh * CH:(nh + 1) * CH],
                            in_=ysb)
                    else:
                        if nh == 0:
                            cell["ysb"] = ypool.tile([128, HIDDEN], f16,
                                                     tag="yb",
                                                     name=f"yb{c}_{it}")
                        ysb = cell["ysb"]
                        nc.vector.tensor_copy(
                            ysb[:, nh * CH:(nh + 1) * CH], yp)
                        if nh == NCH - 1:
                            nc.gpsimd.dma_start(out=y_d[r0:r0 + 128, :],
                                                in_=ysb)
                return f

            def enqueue_y(c):
                for it in range(CH // 128):
                    cell = {}
                    for nh in range(NCH):
                        for h in range(GROUP):
                            fill.append(make_yh(c, it, nh, h, cell))

            def load_masks(c):
                for b in range(nblocks(c)):
                    mt = proj.tile([128, CH], f16, tag=f"m{b}", name=f"m{b}_{c}")
                    nc.sync.dma_start(
                        out=mt,
                        in_=mk_d[b * 128:(b + 1) * 128, c * CH:(c + 1) * CH])
                    mask_sb[b] = mt

            # ---- top-level emission ----
            chain = None
            if mode == "causal":
                for c in range(NCH):
                    p1 = {2: chain[0]} if chain is not None else {}
                    p2 = {0: chain[1]} if chain is not None else {}
                    proj_pass(c, ["k", "v", 0, 1], p1)
                    vtrans(c)
                    proj_pass(c, [2, 3], p2, fill_rate=1)
                    chain = None
                    if c < NCH - 1:
                        drain_fill()
                    if c > 0:
                        enqueue_y(c - 1)
                    for h in range(GROUP):
                        chain = attn_head(c, h, chain)
            else:
                for c in range(NCH):
                    proj_pass(c, ["k", "v", 0, 1], {})
                    vtrans(c)
                    proj_pass(c, [2, 3], {})
                for c in range(NCH):
                    if mode == "generic":
                        load_masks(c)
                    if c > 0:
                        enqueue_y(c - 1)
                    for h in range(GROUP):
                        chain = attn_head(c, h, chain)
            pump(2)
            chain[0]()
            chain[1]()
            drain_fill()
            # Last chunk's y tiles: open the first four psum groups with the
            # h3 matmul deferred, so the PE has ~3us of ready work while the
            # last head's colsum->recip->broadcast->normalize chain drains.
            open_cells = [{} for _ in range(NCH)]
            for nh in range(NCH):
                for h in range(GROUP - 1):
                    make_yh(NCH - 1, 0, nh, h, open_cells[nh])()
            for nh in range(NCH):
                make_yh(NCH - 1, 0, nh, GROUP - 1, open_cells[nh])()
            for it in range(1, CH // 128):
                cell = {}
                for nh in range(NCH):
                    for h in range(GROUP):
                        make_yh(NCH - 1, it, nh, h, cell)()
    nc.finalize()
    return nc


def _get_prog(mode):
    if mode not in _PROG_CACHE:
        _PROG_CACHE[mode] = _build(mode)
    return _PROG_CACHE[mode]


def kernel(x, mask, wq, bq, wk, bk, wv, bv, wo, bo):
    global LAST_EXEC_NS, LAST_RESULTS
    from concourse.bass_utils import run_bass_kernel_spmd

    x = np.asarray(x, dtype=np.float32)
    mask = np.asarray(mask)
    wq = np.asarray(wq, dtype=np.float32)
    bq = np.asarray(bq, dtype=np.float32)
    wk = np.asarray(wk, dtype=np.float32)
    bk = np.asarray(bk, dtype=np.float32)
    wv = np.asarray(wv, dtype=np.float32)
    bv = np.asarray(bv, dtype=np.float32)
    wo = np.asarray(wo, dtype=np.float32)
    bo = np.asarray(bo, dtype=np.float32)

    m2 = mask[0, 0]
    if np.array_equal(m2 != 0, np.tril(np.ones((S, S), dtype=bool))):
        mode = "causal"
    elif np.all(m2 != 0):
        mode = "full"
    else:
        mode = "generic"

    f16 = np.float16
    xT = np.ascontiguousarray(x[0].T).astype(f16)
    xB = np.ascontiguousarray(xT[:, CH:])
    if mode == "causal":
        mstrip = np.triu(np.ones((128, 128), dtype=f16))
    in_maps = []
    for c in range(NCORES):
        qs = slice(4 * c * 128, (4 * c + 4) * 128)
        ks = slice(c * 128, (c + 1) * 128)
        biasp = np.zeros((128, 6), np.float32)
        biasp[:, 0:4] = (bq[qs] * INV_SQRT_D).reshape(4, 128).T
        biasp[:, 4] = bk[ks]
        biasp[:, 5] = bv[ks]
        pk = np.concatenate(
            [wq[:, qs].astype(f16), wk[:, ks].astype(f16),
             wv[:, ks].astype(f16), xT[:, 0:CH]], axis=1)
        im = {
            "pk": np.ascontiguousarray(pk),
            "xB": xB,
            "wo": np.ascontiguousarray(wo[qs, :]).astype(f16),
            "biasp": biasp,
        }
        if mode == "causal":
            im["mstrip"] = mstrip
        if mode == "generic":
            im["maskT"] = np.ascontiguousarray((m2 != 0).T).astype(f16)
        in_maps.append(im)

    nc = _get_prog(mode)
    res = run_bass_kernel_spmd(nc, in_maps, list(range(NCORES)), trace=TRACE)
    LAST_EXEC_NS = res.exec_time_ns
    LAST_RESULTS = res
    y = np.zeros((S, HIDDEN), np.float64)
    for r in res.results:
        y += r["y"].astype(np.float64)
    y = (y + bo.astype(np.float64)).astype(np.float32)
    return y[None]
